# revision 1
# baseline (speedup 1.0000x reference)
"""DeltaNet Trainium2 kernel — 8-core SPMD, one (batch, head) pair per core.

Full inputs -> shard on host -> Bass/Tile kernel per core -> host unshard.

Per-core pipeline (b = core//4, h = core%4):
  xt = X[b]^T resident in SBUF; q/k/v head projections fused with causal
  conv (diagonal-stationary matmuls) and SiLU; l2-norm and beta folded into
  per-token scalars (alpha_q deferred to output, alpha_k/beta folded into
  row scales of the chunked delta-rule); chunked delta rule (C=128) with the
  (I+A)^-1 triangular solve applied via a depth-4 Neumann product directly
  to the rhs; per-chunk RMS-norm and partial o_proj.  Host sums the 4
  per-head partial o_proj outputs per batch.
"""

import os
import sys
from contextlib import ExitStack

import numpy as np

for _p in ("/opt/trn_rl_repo", "/root/.axon_site/_ro/trn_rl_repo"):
    if os.path.isdir(_p) and _p not in sys.path:
        sys.path.insert(0, _p)

import concourse.bass as bass  # noqa: E402
import concourse.tile as tile  # noqa: E402
from concourse import bacc, mybir  # noqa: E402
from concourse.bass_utils import run_bass_kernel_spmd  # noqa: E402

F32 = mybir.dt.float32
F32R = mybir.dt.float32r
AF = mybir.ActivationFunctionType
OP = mybir.AluOpType

HID = 1024
D = 256
C = 128
KT = HID // 128  # 8 k-tiles over the hidden contraction dim
NH = 4
B = 2
S_FULL = 2048

# schedule-tuning knobs (pool buffer counts)
NCHUNK_EMIT = None  # None = all; 0 = skip chunk loop
PASS2 = True

TUNE = {
    "pcs": 2, "pcm": 2, "pS": 2, "pout": 2, "ptok": 2,
    "ppop": 1, "ppt": 5, "praw": 2, "pw": 1, "pdiag": 1, "psq": 3,
}


def build_nc(nchunk=S_FULL // C, dbg=False, reps=1):
    S = nchunk * C
    scs = 512 if S >= 512 else S
    nsc = S // scs
    nc = bacc.Bacc("TRN2", target_bir_lowering=False, debug=False)

    xt_d = nc.dram_tensor("xt", [HID, S], F32R, kind="ExternalInput")
    wq_d = nc.dram_tensor("wq", [HID, D], F32R, kind="ExternalInput")
    wk_d = nc.dram_tensor("wk", [HID, D], F32R, kind="ExternalInput")
    wv_d = nc.dram_tensor("wv", [HID, D], F32R, kind="ExternalInput")
    wb_d = nc.dram_tensor("wb", [HID, 1], F32R, kind="ExternalInput")
    wo_d = nc.dram_tensor("wo", [D, HID], F32R, kind="ExternalInput")
    cdq_d = nc.dram_tensor("cdq", [128, 8 * 128], F32R, kind="ExternalInput")
    cdk_d = nc.dram_tensor("cdk", [128, 8 * 128], F32R, kind="ExternalInput")
    cdv_d = nc.dram_tensor("cdv", [128, 8 * 128], F32R, kind="ExternalInput")
    ident_d = nc.dram_tensor("ident", [128, 128], F32R, kind="ExternalInput")
    onescol_d = nc.dram_tensor("onescol", [128, 1], F32R, kind="ExternalInput")
    mlow_d = nc.dram_tensor("mlow", [128, 128], F32, kind="ExternalInput")
    mup_d = nc.dram_tensor("mup", [128, 128], F32, kind="ExternalInput")
    out_d = nc.dram_tensor("out", [S, HID], F32, kind="ExternalOutput")
    dbg_d = {}
    if dbg:
        for t in ("q", "k", "v"):
            for dt_ in range(2):
                dbg_d[f"{t}{dt_}"] = nc.dram_tensor(
                    f"dbg_{t}{dt_}", [128, S], F32, kind="ExternalOutput"
                )
        for r in ("bk", "nbk2", "aq"):
            dbg_d[r] = nc.dram_tensor(f"dbg_{r}", [1, S], F32, kind="ExternalOutput")
        dbg_d["tok"] = nc.dram_tensor("dbg_tok", [128, 4 * nchunk], F32, kind="ExternalOutput")
        dbg_d["w"] = nc.dram_tensor("dbg_w", [128, 256 * nchunk], F32, kind="ExternalOutput")
        dbg_d["on"] = nc.dram_tensor("dbg_on", [128, 256 * nchunk], F32, kind="ExternalOutput")
        dbg_d["X"] = nc.dram_tensor("dbg_X", [128, 128 * nchunk], F32, kind="ExternalOutput")
        dbg_d["S"] = nc.dram_tensor("dbg_S", [128, 512 * nchunk], F32, kind="ExternalOutput")

    with tile.TileContext(nc) as tc, ExitStack() as ctx:
        # ---------------- persistent pools ----------------
        pmask = ctx.enter_context(tc.tile_pool(name="pmask", bufs=1))
        prow = ctx.enter_context(tc.tile_pool(name="prow", bufs=1))
        pplane = ctx.enter_context(tc.tile_pool(name="pplane", bufs=1))
        pwo = ctx.enter_context(tc.tile_pool(name="pwo", bufs=1))

        ident = pmask.tile([128, 128], F32R)
        onescol = pmask.tile([128, 1], F32R)
        mlow = pmask.tile([128, 128], F32)
        mup = pmask.tile([128, 128], F32)
        eps6 = pmask.tile([128, 1], F32)
        nc.vector.memset(eps6, 1e-6)
        eps5 = pmask.tile([128, 1], F32)
        nc.vector.memset(eps5, 1e-5)

        # packed per-token scale rows {bk, nbk2, aq} for per-chunk transposes
        rows4 = prow.tile([4, S], F32R)

        wo_sb = pwo.tile([128, 2, HID], F32R)

        # q/k/v planes, 2 d-tiles each (post conv+silu, raw scales)
        planes = {}
        for t in ("q", "k", "v"):
            for dt_ in range(2):
                planes[(t, dt_)] = pplane.tile(
                    [128, S], F32R, tag=f"plane_{t}{dt_}", name=f"plane_{t}{dt_}"
                )

        for _rep in range(reps):
            # ---------------- phase B: projections + conv + silu + beta --------
            with ExitStack() as bc2:
                pbc = bc2.enter_context(tc.tile_pool(name="pbc", bufs=1))
                beta_row = pbc.tile([1, S], F32R)
                ak_row = pbc.tile([1, S], F32R)
                bk_row = pbc.tile([1, S], F32R)    # beta * alpha_k
                nbk2_row = pbc.tile([1, S], F32R)  # -beta * alpha_k^2
                aq_row = pbc.tile([1, S], F32R)    # alpha_q

                with ExitStack() as bctx:
                    pxt = bctx.enter_context(tc.tile_pool(name="pxt", bufs=1))
                    pw = bctx.enter_context(tc.tile_pool(name="pw", bufs=TUNE["pw"]))
                    pdiag = bctx.enter_context(tc.tile_pool(name="pdiag", bufs=TUNE["pdiag"]))
                    praw = bctx.enter_context(tc.tile_pool(name="praw", bufs=TUNE["praw"]))
                    ppt_b = bctx.enter_context(
                        tc.tile_pool(name="pptb", bufs=5, space="PSUM")
                    )

                    wb_sb = pw.tile([128, KT, 1], F32R, tag="wb")
                    nc.sync.dma_start(
                        out=wb_sb, in_=wb_d.ap().rearrange("(k p) o -> p k o", p=128)
                    )
                    wd_srcs = {"q": wq_d, "k": wk_d, "v": wv_d}
                    w_sbs = {}
                    w_sbs["q"] = pw.tile([128, KT, D], F32R, tag="w", name="w_q", bufs=2)
                    nc.sync.dma_start(
                        out=w_sbs["q"],
                        in_=wd_srcs["q"].ap().rearrange("(k p) d -> p k d", p=128),
                    )
                    xt_sb = pxt.tile([128, KT, S], F32R)
                    xt_src = xt_d.ap().rearrange("(k p) s -> p k s", p=128)
                    for kk in range(KT):
                        nc.sync.dma_start(out=xt_sb[:, kk, :], in_=xt_src[:, kk, :])

                    # beta row
                    for sc in range(nsc):
                        psb = ppt_b.tile([1, scs], F32, tag="ps", name="psb")
                        for kk in range(KT):
                            nc.tensor.matmul(
                                psb,
                                wb_sb[:, kk, :],
                                xt_sb[:, kk, sc * scs : (sc + 1) * scs],
                                start=(kk == 0),
                                stop=(kk == KT - 1),
                            )
                        nc.scalar.activation(
                            out=beta_row[0:1, sc * scs : (sc + 1) * scs],
                            in_=psb,
                            func=AF.Sigmoid,
                        )

                    copy_flip = 0
                    for t in ("q", "k", "v"):
                        cd_d = {"q": cdq_d, "k": cdk_d, "v": cdv_d}[t]
                        if t not in w_sbs:
                            w_sbs[t] = pw.tile(
                                [128, KT, D], F32R, tag="w", name=f"w_{t}", bufs=2
                            )
                            nc.sync.dma_start(
                                out=w_sbs[t],
                                in_=wd_srcs[t].ap().rearrange(
                                    "(k p) d -> p k d", p=128
                                ),
                            )
                        w_sb = w_sbs[t]
                        diag = pdiag.tile(
                            [128, 8 * 128], F32R, tag="diag", name=f"diag_{t}"
                        )
                        nc.sync.dma_start(out=diag, in_=cd_d.ap())
                        for dt_ in range(2):
                            raw = praw.tile(
                                [128, S + 8], F32R, tag="raw", name=f"raw_{t}{dt_}"
                            )
                            nc.gpsimd.memset(raw[:, 0:8].bitcast(F32), 0.0)
                            for sc in range(nsc):
                                ps = ppt_b.tile([128, scs], F32, tag="ps", name="psraw")
                                for kk in range(KT):
                                    nc.tensor.matmul(
                                        ps,
                                        w_sb[:, kk, dt_ * 128 : (dt_ + 1) * 128],
                                        xt_sb[:, kk, sc * scs : (sc + 1) * scs],
                                        start=(kk == 0),
                                        stop=(kk == KT - 1),
                                    )
                                dst = raw[:, 8 + sc * scs : 8 + (sc + 1) * scs]
                                if copy_flip % 2 == 0:
                                    nc.scalar.activation(out=dst, in_=ps, func=AF.Copy)
                                else:
                                    nc.vector.tensor_copy(dst, ps)
                                copy_flip += 1
                            # conv (4 taps as diagonal-stationary matmuls) + SiLU
                            plane = planes[(t, dt_)]
                            for sc in range(nsc):
                                base = sc * scs
                                psc = ppt_b.tile([128, scs], F32, tag="ps", name="psconv")
                                for j in (3, 2, 1, 0):
                                    sh = 3 - j
                                    dslc = diag[
                                        :, (j * 2 + dt_) * 128 : (j * 2 + dt_ + 1) * 128
                                    ]
                                    nc.tensor.matmul(
                                        psc,
                                        dslc,
                                        raw[:, 8 + base - sh : 8 + base + scs - sh],
                                        start=(j == 3),
                                        stop=(j == 0),
                                    )
                                nc.scalar.activation(
                                    out=plane[:, base : base + scs], in_=psc, func=AF.Silu
                                )

                # deferred small DMAs (keep the SP ring clear for xt at startup)
                nc.sync.dma_start(out=ident, in_=ident_d.ap())
                nc.sync.dma_start(out=onescol, in_=onescol_d.ap())
                nc.sync.dma_start(out=mlow, in_=mlow_d.ap())
                nc.sync.dma_start(out=mup, in_=mup_d.ap())
                nc.sync.dma_start(
                    out=wo_sb, in_=wo_d.ap().rearrange("(t p) h -> p t h", p=128)
                )

                # ---------------- phase C: l2 rows ----------------
                with ExitStack() as cctx:
                    psq = cctx.enter_context(tc.tile_pool(name="psq", bufs=TUNE["psq"]))
                    pscr = cctx.enter_context(tc.tile_pool(name="pscr", bufs=2))
                    ppt_c = cctx.enter_context(
                        tc.tile_pool(name="pptc", bufs=4, space="PSUM")
                    )
                    for t, dest in (("q", aq_row), ("k", ak_row)):
                        for sc in range(nsc):
                            sl = slice(sc * scs, (sc + 1) * scs)
                            psl = ppt_c.tile([1, scs], F32, tag="ps", name="psl")
                            for dt_ in range(2):
                                sq = psq.tile([128, scs], F32R, tag="sq", name="sq")
                                src = planes[(t, dt_)][:, sl]
                                if dt_ == 0:
                                    nc.vector.tensor_mul(sq, src, src)
                                else:
                                    nc.scalar.activation(out=sq, in_=src, func=AF.Square)
                                nc.tensor.matmul(
                                    psl, onescol, sq, start=(dt_ == 0), stop=(dt_ == 1)
                                )
                            scr = pscr.tile([1, scs], F32, tag="scr", name="scr")
                            nc.scalar.activation(
                                out=scr, in_=psl, func=AF.Sqrt, bias=eps6[0:1, :]
                            )
                            with nc.allow_low_precision(reason="f32r row scales"):
                                nc.vector.reciprocal(out=dest[0:1, sl], in_=scr)
                            if t == "q":
                                # pack alpha_q slice immediately
                                nc.sync.dma_start(
                                    out=rows4[2:3, sl], in_=aq_row[0:1, sl]
                                )
                            else:
                                # bk/nbk2 slices as soon as alpha_k slice lands
                                with nc.allow_low_precision(reason="f32r rows"):
                                    nc.vector.tensor_mul(
                                        bk_row[0:1, sl],
                                        beta_row[0:1, sl],
                                        ak_row[0:1, sl],
                                    )
                                    nc.vector.scalar_tensor_tensor(
                                        out=nbk2_row[0:1, sl],
                                        in0=bk_row[0:1, sl],
                                        scalar=-1.0,
                                        in1=ak_row[0:1, sl],
                                        op0=OP.mult,
                                        op1=OP.mult,
                                    )
                                nc.sync.dma_start(
                                    out=rows4[0:1, sl], in_=bk_row[0:1, sl]
                                )
                                nc.sync.dma_start(
                                    out=rows4[1:2, sl], in_=nbk2_row[0:1, sl]
                                )

                if dbg:
                    for t in ("q", "k", "v"):
                        for dt_ in range(2):
                            nc.sync.dma_start(
                                out=dbg_d[f"{t}{dt_}"].ap(),
                                in_=planes[(t, dt_)][:, :].bitcast(F32),
                            )
                    nc.sync.dma_start(out=dbg_d["bk"].ap(), in_=bk_row[0:1, :].bitcast(F32))
                    nc.sync.dma_start(
                        out=dbg_d["nbk2"].ap(), in_=nbk2_row[0:1, :].bitcast(F32)
                    )
                    nc.sync.dma_start(out=dbg_d["aq"].ap(), in_=aq_row[0:1, :].bitcast(F32))



            # ---------------- phase D: chunked delta rule ----------------
            dctx = ExitStack()
            pS = dctx.enter_context(tc.tile_pool(name="pS", bufs=TUNE["pS"]))
            pcs = dctx.enter_context(tc.tile_pool(name="pcs", bufs=TUNE["pcs"]))
            pcm = dctx.enter_context(tc.tile_pool(name="pcm", bufs=TUNE["pcm"]))
            ptok = dctx.enter_context(tc.tile_pool(name="ptok", bufs=TUNE["ptok"]))
            pout = dctx.enter_context(tc.tile_pool(name="pout", bufs=TUNE["pout"]))
            ppS = dctx.enter_context(tc.tile_pool(name="ppS", bufs=1, space="PSUM"))
            ppop = dctx.enter_context(tc.tile_pool(name="ppop", bufs=TUNE["ppop"], space="PSUM"))
            ppt = dctx.enter_context(tc.tile_pool(name="ppt", bufs=TUNE["ppt"], space="PSUM"))

            psS0 = ppS.tile([128, 256], F32, tag="psS0", name="psS0")
            psS1 = ppS.tile([128, 256], F32, tag="psS1", name="psS1")

            _emit_n = nchunk if NCHUNK_EMIT is None else min(NCHUNK_EMIT, nchunk)

            # ---- pass 1: chunk-independent work (parallel across chunks) ----
            tok_l, Z_l, Z2_l, Z4_l, Z8_l, Hm_l, vb_l, ktok_l = (
                [None] * _emit_n for _ in range(8)
            )

            def emit_pass1(i):
                ch = slice(i * C, (i + 1) * C)
                k0 = planes[("k", 0)][:, ch]
                k1 = planes[("k", 1)][:, ch]
                q0 = planes[("q", 0)][:, ch]
                q1 = planes[("q", 1)][:, ch]

                # token scalars -> [128, {bk, nbk2, aq}]
                psR = ppt.tile([128, 4], F32R, tag="ps", name="psR")
                nc.tensor.transpose(psR[:, 0:4], rows4[0:4, ch], ident[0:4, 0:4])
                tok = ptok.tile([128, 4], F32, tag="tok", name="tok", bufs=_emit_n)
                nc.vector.tensor_copy(tok[:, 0:3], psR[:, 0:3])
                tok_l[i] = tok
                if dbg:
                    nc.sync.dma_start(
                        out=dbg_d["tok"].ap()[:, i * 4 : (i + 1) * 4], in_=tok
                    )
                nbk2_t = tok[:, 1:2]

                # A'' = Kc Kc^T ; X = (A''*nbk2) * mlow
                psA = ppt.tile([128, 128], F32, tag="ps", name="psA")
                nc.tensor.matmul(psA, k0, k0, start=True, stop=False)
                nc.tensor.matmul(psA, k1, k1, start=False, stop=True)
                X = pcs.tile([128, 128], F32R, tag="X", name="X")
                nc.vector.scalar_tensor_tensor(
                    out=X, in0=psA, scalar=nbk2_t, in1=mlow, op0=OP.mult, op1=OP.mult
                )
                psZ = ppt.tile([128, 128], F32R, tag="ps", name="psZ")
                nc.tensor.transpose(psZ, X, ident)
                Z = pcs.tile([128, 128], F32R, tag="Z", name="Z", bufs=_emit_n)
                nc.scalar.activation(out=Z, in_=psZ, func=AF.Copy)
                Z_l[i] = Z

                def sqmm(lhsT, rhs, name, eng, keep):
                    psp = ppt.tile([128, 128], F32, tag="ps", name=f"psp_{name}")
                    nc.tensor.matmul(psp, lhsT, rhs, start=True, stop=True)
                    t_ = pcs.tile(
                        [128, 128],
                        F32R,
                        tag=name,
                        name=name,
                        bufs=(_emit_n if keep else 2),
                    )
                    if eng == "s":
                        nc.scalar.activation(out=t_, in_=psp, func=AF.Copy)
                    else:
                        nc.vector.tensor_copy(t_, psp)
                    return t_

                X2 = sqmm(Z, X, "X2", "s", False)
                Z2 = sqmm(X, Z, "Z2", "v", True)
                X4 = sqmm(Z2, X2, "X4", "s", False)
                Z4 = sqmm(X2, Z2, "Z4", "v", True)
                Z8 = sqmm(X4, Z4, "Z8", "s", True)
                Z2_l[i], Z4_l[i], Z8_l[i] = Z2, Z4, Z8

                # V transpose + vb = bk * v_tok
                psV = ppt.tile([128, 256], F32R, tag="ps", name="psV")
                nc.tensor.transpose(psV[:, 0:128], planes[("v", 0)][:, ch], ident)
                nc.tensor.transpose(psV[:, 128:256], planes[("v", 1)][:, ch], ident)
                vb = pcm.tile([128, 256], F32R, tag="vb", name="vb", bufs=_emit_n)
                nc.vector.tensor_scalar(
                    out=vb, in0=psV, scalar1=tok[:, 0:1], scalar2=None, op0=OP.mult
                )
                vb_l[i] = vb

                # H~m = (K Q^T) * mup
                psH = ppt.tile([128, 128], F32, tag="ps", name="psH")
                nc.tensor.matmul(psH, k0, q0, start=True, stop=False)
                nc.tensor.matmul(psH, k1, q1, start=False, stop=True)
                Hm = pcs.tile([128, 128], F32R, tag="Hm", name="Hm", bufs=_emit_n)
                nc.vector.tensor_mul(Hm, psH, mup)
                Hm_l[i] = Hm

                # ktok (K chunk transposed)
                psK = ppt.tile([128, 256], F32R, tag="ps", name="psK")
                nc.tensor.transpose(psK[:, 0:128], k0, ident)
                nc.tensor.transpose(psK[:, 128:256], k1, ident)
                ktok = pcm.tile([128, 256], F32R, tag="ktok", name="ktok", bufs=_emit_n)
                nc.scalar.activation(out=ktok, in_=psK, func=AF.Copy)
                ktok_l[i] = ktok

            # ---- pass 2, part A: sequential state chain ----
            chain_state = {}

            def emit_chain(i):
                ch = slice(i * C, (i + 1) * C)
                k0 = planes[("k", 0)][:, ch]
                k1 = planes[("k", 1)][:, ch]
                q0 = planes[("q", 0)][:, ch]
                q1 = planes[("q", 1)][:, ch]
                tok = tok_l[i]
                bk_t = tok[:, 0:1]
                nbk2_t = tok[:, 1:2]
                aq_t = tok[:, 2:3]
                vb = vb_l[i]
                Hm = Hm_l[i]
                ktok = ktok_l[i]

                if i > 0:
                    S_sb = pS.tile([128, 512], F32R, tag="S", name="S_sb")
                    nc.scalar.activation(out=S_sb[:, 0:256], in_=psS0, func=AF.Copy)
                    nc.scalar.activation(out=S_sb[:, 256:512], in_=psS1, func=AF.Copy)

                # y = vb + nbk2 * (K S)
                if i > 0:
                    psKS = ppt.tile([128, 256], F32, tag="ps", name="psKS")
                    nc.tensor.matmul(psKS, k0, S_sb[:, 0:256], start=True, stop=False)
                    nc.tensor.matmul(
                        psKS, k1, S_sb[:, 256:512], start=False, stop=True
                    )
                    y = pcm.tile([128, 256], F32R, tag="y", name="y")
                    nc.vector.scalar_tensor_tensor(
                        out=y, in0=psKS, scalar=nbk2_t, in1=vb, op0=OP.mult, op1=OP.add
                    )
                else:
                    y = vb

                # t-chain: w = (I+X)(I+X2)(I+X4)(I+X8) y
                cur = y
                for idx3, P in enumerate((Z8_l[i], Z4_l[i], Z2_l[i], Z_l[i])):
                    pst = ppt.tile([128, 256], F32, tag="ps", name=f"pst{idx3}")
                    nc.tensor.matmul(pst, P, cur, start=True, stop=True)
                    nxt = pcm.tile([128, 256], F32R, tag=f"t{idx3}", name=f"t{idx3}")
                    nc.vector.tensor_add(nxt, pst, cur)
                    cur = nxt
                w = cur

                if dbg:
                    nc.sync.dma_start(
                        out=dbg_d["w"].ap()[:, i * 256 : (i + 1) * 256],
                        in_=w[:, :].bitcast(F32),
                    )
                    if i > 0:
                        nc.sync.dma_start(
                            out=dbg_d["S"].ap()[:, i * 512 : (i + 1) * 512],
                            in_=S_sb[:, :].bitcast(F32),
                        )

                # state update S += K^T w  (before o so the chain keeps moving)
                nc.tensor.matmul(
                    psS0,
                    ktok[:, 0:128],
                    w,
                    start=(i == 0),
                    stop=(i == _emit_n - 1),
                    skip_group_check=True,
                )
                nc.tensor.matmul(
                    psS1,
                    ktok[:, 128:256],
                    w,
                    start=(i == 0),
                    stop=(i == _emit_n - 1),
                    skip_group_check=True,
                )

                # o = Q S + Hm^T w
                pso = ppt.tile([128, 256], F32, tag="ps", name="pso")
                if i > 0:
                    nc.tensor.matmul(pso, q0, S_sb[:, 0:256], start=True, stop=False)
                    nc.tensor.matmul(
                        pso, q1, S_sb[:, 256:512], start=False, stop=False
                    )
                    nc.tensor.matmul(pso, Hm, w, start=False, stop=True)
                else:
                    nc.tensor.matmul(pso, Hm, w, start=True, stop=True)

                chain_state[i] = (pso, S_sb if i > 0 else None, w)

            # ---- pass 2, part B: trailing per-chunk output work ----
            def emit_trailing(i):
                ch = slice(i * C, (i + 1) * C)
                tok = tok_l[i]
                aq_t = tok[:, 2:3]
                pso, S_sb, w = chain_state.pop(i)

                # rms-norm on alpha_q-scaled o
                sums = ptok.tile([128, 1], F32, tag="sums", name="sums")
                scratch = pcm.tile([128, 256], F32, tag="scr", name="scratch")
                nc.scalar.activation(
                    out=scratch, in_=pso, func=AF.Square, accum_out=sums
                )
                aq2 = ptok.tile([128, 1], F32, tag="aq2", name="aq2")
                nc.vector.tensor_scalar(
                    out=aq2,
                    in0=aq_t,
                    scalar1=aq_t,
                    scalar2=1.0 / D,
                    op0=OP.mult,
                    op1=OP.mult,
                )
                rstd_t = ptok.tile([128, 1], F32, tag="rstd_t", name="rstd_t")
                nc.scalar.activation(
                    out=rstd_t, in_=sums, func=AF.Sqrt, scale=aq2[:, 0:1], bias=eps5
                )
                rstd = ptok.tile([128, 1], F32, tag="rstd", name="rstd")
                nc.vector.reciprocal(out=rstd, in_=rstd_t)
                on = pcm.tile([128, 256], F32R, tag="on", name="on")
                nc.vector.tensor_scalar(
                    out=on,
                    in0=pso,
                    scalar1=rstd[:, 0:1],
                    scalar2=aq_t,
                    op0=OP.mult,
                    op1=OP.mult,
                )
                if dbg:
                    nc.sync.dma_start(
                        out=dbg_d["on"].ap()[:, i * 256 : (i + 1) * 256],
                        in_=on[:, :].bitcast(F32),
                    )

                # o transpose + partial o_proj
                psOT = ppt.tile([128, 256], F32R, tag="ps", name="psOT")
                nc.tensor.transpose(psOT[:, 0:128], on[:, 0:128], ident)
                nc.tensor.transpose(psOT[:, 128:256], on[:, 128:256], ident)
                ot = pcm.tile([128, 256], F32R, tag="ot", name="ot")
                nc.scalar.activation(out=ot, in_=psOT, func=AF.Copy)
                outbuf = pout.tile([128, HID], F32, tag="outbuf", name="outbuf")
                for hc in range(2):
                    psop = ppop.tile([128, 512], F32, tag="op", name="psop")
                    nc.tensor.matmul(
                        psop,
                        ot[:, 0:128],
                        wo_sb[:, 0, hc * 512 : (hc + 1) * 512],
                        start=True,
                        stop=False,
                    )
                    nc.tensor.matmul(
                        psop,
                        ot[:, 128:256],
                        wo_sb[:, 1, hc * 512 : (hc + 1) * 512],
                        start=False,
                        stop=True,
                    )
                    if hc == 0:
                        nc.vector.tensor_copy(outbuf[:, 0:512], psop)
                    else:
                        nc.scalar.activation(
                            out=outbuf[:, 512:1024], in_=psop, func=AF.Copy
                        )
                nc.sync.dma_start(out=out_d.ap()[ch, :], in_=outbuf)

            LOOKAHEAD = 2
            for i in range(min(LOOKAHEAD, _emit_n)):
                emit_pass1(i)
            for i in range(_emit_n):
                if PASS2:
                    emit_chain(i)
                if i + LOOKAHEAD < _emit_n:
                    emit_pass1(i + LOOKAHEAD)
                if PASS2 and i > 0:
                    emit_trailing(i - 1)
            if PASS2:
                emit_trailing(_emit_n - 1)
            dctx.close()

    nc.compile()
    return nc


def make_host_inputs(inputs, nchunk=S_FULL // C):
    """Shard + preprocess full inputs into per-core in_maps."""
    S = nchunk * C
    hs = np.ascontiguousarray(np.asarray(inputs["hidden_states"])[:, :S, :]).astype(
        np.float32
    )
    Wq, Wk, Wv = (np.asarray(inputs[k], np.float32) for k in ("Wq", "Wk", "Wv"))
    Wb = np.asarray(inputs["Wb"], np.float32)
    Wo = np.asarray(inputs["Wo"], np.float32)
    nw = np.asarray(inputs["norm_w"], np.float32)
    convs = {
        k: np.asarray(inputs[k], np.float32) for k in ("conv_q", "conv_k", "conv_v")
    }

    ident = np.eye(128, dtype=np.float32)
    onescol = np.ones((128, 1), np.float32)
    mlow = np.tril(np.ones((128, 128), np.float32), -1)
    mup = np.triu(np.ones((128, 128), np.float32), 0)

    def diag_pack(cw):
        # cw: [256, 4] tap weights for this head -> [128, 8*128]
        out = np.zeros((128, 8 * 128), np.float32)
        for j in range(4):
            for dt_ in range(2):
                blk = np.diag(cw[dt_ * 128 : (dt_ + 1) * 128, j])
                out[:, (j * 2 + dt_) * 128 : (j * 2 + dt_ + 1) * 128] = blk
        return out

    in_maps = []
    for core in range(8):
        b, h = core // 4, core % 4
        hsel = slice(h * D, (h + 1) * D)
        in_maps.append(
            {
                "xt": np.ascontiguousarray(hs[b].T),
                "wq": np.ascontiguousarray(Wq[:, hsel]),
                "wk": np.ascontiguousarray(Wk[:, hsel]),
                "wv": np.ascontiguousarray(Wv[:, hsel]),
                "wb": np.ascontiguousarray(Wb[:, h : h + 1]),
                "wo": np.ascontiguousarray(nw[:, None] * Wo[hsel, :]),
                "cdq": diag_pack(convs["conv_q"][hsel]),
                "cdk": diag_pack(convs["conv_k"][hsel]),
                "cdv": diag_pack(convs["conv_v"][hsel]),
                "ident": ident,
                "onescol": onescol,
                "mlow": mlow,
                "mup": mup,
            }
        )
    return in_maps


_NC_CACHE = {}


def _get_nc(nchunk):
    if nchunk not in _NC_CACHE:
        _NC_CACHE[nchunk] = build_nc(nchunk)
    return _NC_CACHE[nchunk]


def kernel(**inputs) -> np.ndarray:
    nchunk = S_FULL // C
    nc = _get_nc(nchunk)
    in_maps = make_host_inputs(inputs, nchunk)
    res = run_bass_kernel_spmd(nc, in_maps, core_ids=list(range(8)))
    S = nchunk * C
    out = np.zeros((B, S, HID), np.float32)
    for core in range(8):
        out[core // 4] += res.results[core]["out"]
    return out



# revision 19
# speedup vs baseline: 1.1318x; 1.1318x over previous
"""DeltaNet Trainium2 kernel — 8-core SPMD, one (batch, head) pair per core.

v2: bf16 datapath (fp32 PSUM accumulation), chunked delta rule (C=128) with a
per-chunk UT-transform matrix T = (I+X)(I+X^2)(I+X^4) precomputed off the
serial state chain; beta / l2-norm row sums as N=1 transposed matmuls; k and q
planes interleaved per d-tile so K*K^T and K*Q^T come out of one matmul pair;
RMS-norm sums via matmul on the transposed o with the rstd*alpha_q scaling
folded into the o_proj PSUM-drain copies.  Host folds norm_w into Wo, sums the
4 per-head partial o_proj outputs per batch.
"""

import os
import sys
from contextlib import ExitStack

import ml_dtypes
import numpy as np

for _p in ("/opt/trn_rl_repo", "/root/.axon_site/_ro/trn_rl_repo"):
    if os.path.isdir(_p) and _p not in sys.path:
        sys.path.insert(0, _p)

import concourse.bass as bass  # noqa: E402
import concourse.tile as tile  # noqa: E402
from concourse import bacc, mybir  # noqa: E402
from concourse.bass_utils import run_bass_kernel_spmd  # noqa: E402

F32 = mybir.dt.float32
BF16 = mybir.dt.bfloat16
AF = mybir.ActivationFunctionType
OP = mybir.AluOpType

HID = 1024
D = 256
C = 128
KT = HID // 128
NH = 4
B = 2
S_FULL = 2048
LOOKAHEAD = 2


def build_nc(nchunk=S_FULL // C, dbg=False):
    S = nchunk * C
    scs = 512 if S >= 512 else S
    nsc = S // scs
    nc = bacc.Bacc("TRN2", target_bir_lowering=False, debug=False)

    xt_d = nc.dram_tensor("xt", [HID, S], BF16, kind="ExternalInput")
    wq_d = nc.dram_tensor("wq", [HID, D], BF16, kind="ExternalInput")
    wk_d = nc.dram_tensor("wk", [HID, D], BF16, kind="ExternalInput")
    wv_d = nc.dram_tensor("wv", [HID, D], BF16, kind="ExternalInput")
    wb_d = nc.dram_tensor("wb", [HID, 1], BF16, kind="ExternalInput")
    wo_d = nc.dram_tensor("wo", [D, HID], BF16, kind="ExternalInput")
    cdq_d = nc.dram_tensor("cdq", [128, 8 * 128], BF16, kind="ExternalInput")
    cdk_d = nc.dram_tensor("cdk", [128, 8 * 128], BF16, kind="ExternalInput")
    cdv_d = nc.dram_tensor("cdv", [128, 8 * 128], BF16, kind="ExternalInput")
    identb_d = nc.dram_tensor("identb", [128, 128], BF16, kind="ExternalInput")
    onescol_d = nc.dram_tensor("onescol", [128, 1], BF16, kind="ExternalInput")
    mlow_d = nc.dram_tensor("mlow", [128, 128], F32, kind="ExternalInput")
    mup_d = nc.dram_tensor("mup", [128, 128], F32, kind="ExternalInput")
    out_d = nc.dram_tensor("out", [S, HID], F32, kind="ExternalOutput")
    dbg_d = {}
    if dbg:
        for nm, w_ in (("kq0", 2 * S), ("kq1", 2 * S), ("v0", S), ("v1", S),
                       ("tok", 4 * nchunk), ("X", 128 * nchunk),
                       ("T", 128 * nchunk), ("u", 256 * nchunk),
                       ("Gt", 256 * nchunk), ("ktok", 256 * nchunk),
                       ("w", 256 * nchunk), ("o_sb", 256 * nchunk),
                       ("S_sb", 512 * nchunk)):
            dt_ = F32 if nm == "tok" else BF16
            dbg_d[nm] = nc.dram_tensor(f"dbg_{nm}", [128, w_], dt_,
                                       kind="ExternalOutput")

    with tile.TileContext(nc) as tc, ExitStack() as ctx:
        # ---------------- persistent pools ----------------
        pconst = ctx.enter_context(tc.tile_pool(name="pconst", bufs=1))
        pplane = ctx.enter_context(tc.tile_pool(name="pplane", bufs=1))
        pw = ctx.enter_context(tc.tile_pool(name="pw", bufs=1))
        pxt = ctx.enter_context(tc.tile_pool(name="pxt", bufs=1))

        identb = pconst.tile([128, 128], BF16)
        onescol = pconst.tile([128, 1], BF16)
        mlow = pconst.tile([128, 128], F32)
        mup = pconst.tile([128, 128], F32)
        eps6 = pconst.tile([128, 1], F32)
        eps5 = pconst.tile([128, 1], F32)
        nc.vector.memset(eps6, 1e-6)
        nc.vector.memset(eps5, 1e-5)

        wo_sb = pw.tile([128, 2, HID], BF16)
        wb_sb = pw.tile([128, KT, 1], BF16)

        # planes: kq{dt} holds k in [:,0,:] and q in [:,1,:]; v separate
        kq0 = pplane.tile([128, 2, S], BF16, name="kq0")
        kq1 = pplane.tile([128, 2, S], BF16, name="kq1")
        v0 = pplane.tile([128, S], BF16, name="v0")
        v1 = pplane.tile([128, S], BF16, name="v1")

        xt_sb = pxt.tile([128, KT, S], BF16)

        # ---------------- phase B: projections + conv + silu ----------------
        with ExitStack() as bctx:
            pwt = bctx.enter_context(tc.tile_pool(name="pwt", bufs=2))
            pdiag = bctx.enter_context(tc.tile_pool(name="pdiag", bufs=2))
            praw = bctx.enter_context(tc.tile_pool(name="praw", bufs=2))
            ppb = bctx.enter_context(tc.tile_pool(name="ppb", bufs=5, space="PSUM"))

            nc.sync.dma_start(
                out=wb_sb, in_=wb_d.ap().rearrange("(k p) o -> p k o", p=128)
            )
            wd_srcs = {"q": wq_d, "k": wk_d, "v": wv_d}
            w_sbs = {}
            w_sbs["k"] = pwt.tile([128, KT, D], BF16, tag="w", name="w_k", bufs=3)
            nc.sync.dma_start(
                out=w_sbs["k"],
                in_=wd_srcs["k"].ap().rearrange("(k p) d -> p k d", p=128),
            )
            xt_src = xt_d.ap().rearrange("(k p) s -> p k s", p=128)
            for kk in range(KT):
                nc.sync.dma_start(out=xt_sb[:, kk, :], in_=xt_src[:, kk, :])

            copy_flip = 0
            for t in ("k", "q", "v"):
                cd_d = {"q": cdq_d, "k": cdk_d, "v": cdv_d}[t]
                if t not in w_sbs:
                    w_sbs[t] = pwt.tile(
                        [128, KT, D], BF16, tag="w", name=f"w_{t}", bufs=3
                    )
                    nc.sync.dma_start(
                        out=w_sbs[t],
                        in_=wd_srcs[t].ap().rearrange("(k p) d -> p k d", p=128),
                    )
                w_sb = w_sbs[t]
                diag = pdiag.tile([128, 8 * 128], BF16, tag="diag", name=f"diag_{t}")
                nc.sync.dma_start(out=diag, in_=cd_d.ap())
                for dt_ in range(2):
                    raw = praw.tile(
                        [128, S + 8], BF16, tag="raw", name=f"raw_{t}{dt_}"
                    )
                    nc.gpsimd.memset(raw[:, 0:8], 0.0)
                    for sc in range(nsc):
                        ps = ppb.tile([128, scs], F32, tag="ps", name="psraw")
                        for kk in range(KT):
                            nc.tensor.matmul(
                                ps,
                                w_sb[:, kk, dt_ * 128 : (dt_ + 1) * 128],
                                xt_sb[:, kk, sc * scs : (sc + 1) * scs],
                                start=(kk == 0),
                                stop=(kk == KT - 1),
                            )
                        dst = raw[:, 8 + sc * scs : 8 + (sc + 1) * scs]
                        if copy_flip % 2 == 0:
                            nc.vector.tensor_copy(dst, ps)
                        else:
                            nc.scalar.activation(out=dst, in_=ps, func=AF.Copy)
                        copy_flip += 1
                    # conv (4 taps as diagonal-stationary matmuls) + SiLU
                    if t == "v":
                        pdst = (v0, v1)[dt_]
                        dsts = [
                            pdst[:, sc * scs : (sc + 1) * scs] for sc in range(nsc)
                        ]
                    else:
                        kqp = (kq0, kq1)[dt_]
                        ti = 0 if t == "k" else 1
                        dsts = [
                            kqp[:, ti, sc * scs : (sc + 1) * scs]
                            for sc in range(nsc)
                        ]
                    for sc in range(nsc):
                        base = sc * scs
                        psc = ppb.tile([128, scs], F32, tag="ps", name="psconv")
                        for j in (3, 2, 1, 0):
                            sh = 3 - j
                            dslc = diag[
                                :, (j * 2 + dt_) * 128 : (j * 2 + dt_ + 1) * 128
                            ]
                            nc.tensor.matmul(
                                psc,
                                dslc,
                                raw[:, 8 + base - sh : 8 + base + scs - sh],
                                start=(j == 3),
                                stop=(j == 0),
                            )
                        nc.scalar.activation(out=dsts[sc], in_=psc, func=AF.Silu)

            # deferred small DMAs
            nc.sync.dma_start(out=identb, in_=identb_d.ap())
            nc.sync.dma_start(out=onescol, in_=onescol_d.ap())
            nc.sync.dma_start(out=mlow, in_=mlow_d.ap())
            nc.sync.dma_start(out=mup, in_=mup_d.ap())
            nc.sync.dma_start(
                out=wo_sb, in_=wo_d.ap().rearrange("(t p) h -> p t h", p=128)
            )

        # ---------------- phase D: chunked delta rule ----------------
        dctx = ExitStack()
        WIN = LOOKAHEAD + 2
        pS = dctx.enter_context(tc.tile_pool(name="pS", bufs=2))
        pcs = dctx.enter_context(tc.tile_pool(name="pcs", bufs=2))
        pcm = dctx.enter_context(tc.tile_pool(name="pcm", bufs=2))
        pwin = dctx.enter_context(tc.tile_pool(name="pwin", bufs=WIN))
        ptok = dctx.enter_context(tc.tile_pool(name="ptok", bufs=nchunk))
        pout = dctx.enter_context(tc.tile_pool(name="pout", bufs=2))
        ppS = dctx.enter_context(tc.tile_pool(name="ppS", bufs=1, space="PSUM"))
        ppw = dctx.enter_context(tc.tile_pool(name="ppw", bufs=2, space="PSUM"))
        ppt = dctx.enter_context(tc.tile_pool(name="ppt", bufs=3, space="PSUM"))
        pptok = dctx.enter_context(tc.tile_pool(name="pptok", bufs=1, space="PSUM"))

        psS0 = ppS.tile([128, 256], F32, tag="psS0", name="psS0")
        psS1 = ppS.tile([128, 256], F32, tag="psS1", name="psS1")

        state = {}

        def emit_pass1(i):
            ch = slice(i * C, (i + 1) * C)
            k0, q0 = kq0[:, 0, ch], kq0[:, 1, ch]
            k1, q1 = kq1[:, 0, ch], kq1[:, 1, ch]

            # --- token scalars: beta + l2 sums as N<=1 matmuls ---
            # each multi-matmul accumulation group gets its OWN psum tile:
            # two open groups in one bank corrupt each other when the
            # scheduler interleaves them.  pptok bufs=1 serializes via slot
            # reuse.
            psB = pptok.tile([128, 1], F32, tag="ptk", name="psB")
            for kk in range(KT):
                nc.tensor.matmul(
                    psB,
                    xt_sb[:, kk, ch],
                    wb_sb[:, kk, :],
                    start=(kk == 0),
                    stop=(kk == KT - 1),
                )
            sqs = {}
            for nm, src in (("k0", k0), ("k1", k1), ("q0", q0), ("q1", q1)):
                sq = pcs.tile([128, 128], BF16, tag=f"sq_{nm}",
                              name=f"sq_{nm}", bufs=2)
                nc.vector.tensor_mul(sq, src, src)
                sqs[nm] = sq
            tok = ptok.tile([128, 4], F32, tag="tok", name="tok")
            scr = pcs.tile([128, 4], F32, tag="scr", name="scr", bufs=2)
            nc.scalar.activation(out=scr[:, 0:1], in_=psB, func=AF.Sigmoid)
            for col, (a, b2) in ((1, ("k0", "k1")), (2, ("q0", "q1"))):
                psL = pptok.tile([128, 1], F32, tag="ptk", name=f"psL{col}")
                nc.tensor.matmul(psL, sqs[a], onescol, start=True, stop=False)
                nc.tensor.matmul(psL, sqs[b2], onescol, start=False, stop=True)
                nc.scalar.activation(
                    out=scr[:, col : col + 1], in_=psL, func=AF.Sqrt, bias=eps6
                )
            nc.vector.reciprocal(out=scr[:, 3:4], in_=scr[:, 1:2])  # ak
            nc.vector.reciprocal(out=tok[:, 2:3], in_=scr[:, 2:3])  # aq
            # bk = beta*ak ; nbk2 = -bk*ak ; aq2 = aq^2/D
            nc.vector.tensor_mul(tok[:, 0:1], scr[:, 0:1], scr[:, 3:4])
            nc.vector.scalar_tensor_tensor(
                out=tok[:, 1:2], in0=tok[:, 0:1], scalar=-1.0, in1=scr[:, 3:4],
                op0=OP.mult, op1=OP.mult,
            )
            nc.vector.tensor_scalar(
                out=tok[:, 3:4], in0=tok[:, 2:3], scalar1=tok[:, 2:3],
                scalar2=1.0 / D, op0=OP.mult, op1=OP.mult,
            )
            bk_t, nbk2_t = tok[:, 0:1], tok[:, 1:2]

            # --- A'' | H'' in one psum pair ---
            psAH = ppt.tile([128, 256], F32, tag="ps", name="psAH")
            nc.tensor.matmul(psAH, k0, kq0[:, :, ch], start=True, stop=False)
            nc.tensor.matmul(psAH, k1, kq1[:, :, ch], start=False, stop=True)
            X = pcs.tile([128, 128], BF16, tag="X", name="X", bufs=2)
            nc.vector.scalar_tensor_tensor(
                out=X, in0=psAH[:, 0:128], scalar=nbk2_t, in1=mlow,
                op0=OP.mult, op1=OP.mult,
            )
            Hm = pwin.tile([128, 128], BF16, tag="Hm", name="Hm")
            nc.vector.tensor_mul(Hm, psAH[:, 128:256], mup)

            # --- Neumann depth-3: T = (I+X)(I+X2)(I+X4) ---
            psZ = ppt.tile([128, 128], BF16, tag="ps", name="psZ")
            nc.tensor.transpose(psZ, X, identb)
            Z = pcs.tile([128, 128], BF16, tag="Z", name="Z", bufs=2)
            nc.scalar.activation(out=Z, in_=psZ, func=AF.Copy)
            ZI = pcs.tile([128, 128], BF16, tag="ZI", name="ZI", bufs=2)
            nc.vector.tensor_add(ZI, psZ, identb)

            psX2 = ppt.tile([128, 128], F32, tag="ps", name="psX2")
            nc.tensor.matmul(psX2, Z, X, start=True, stop=True)
            X2 = pcs.tile([128, 128], BF16, tag="X2", name="X2", bufs=2)
            nc.scalar.activation(out=X2, in_=psX2, func=AF.Copy)
            X2I = pcs.tile([128, 128], BF16, tag="X2I", name="X2I", bufs=2)
            nc.vector.tensor_add(X2I, psX2, identb)

            psZ2 = ppt.tile([128, 128], F32, tag="ps", name="psZ2")
            nc.tensor.matmul(psZ2, X, Z, start=True, stop=True)
            Z2 = pcs.tile([128, 128], BF16, tag="Z2", name="Z2", bufs=2)
            nc.scalar.activation(out=Z2, in_=psZ2, func=AF.Copy)

            psX4 = ppt.tile([128, 128], F32, tag="ps", name="psX4")
            nc.tensor.matmul(psX4, Z2, X2, start=True, stop=True)
            X4I = pcs.tile([128, 128], BF16, tag="X4I", name="X4I", bufs=2)
            nc.vector.tensor_add(X4I, psX4, identb)

            psXB = ppt.tile([128, 128], F32, tag="ps", name="psXB")
            nc.tensor.matmul(psXB, ZI, X2I, start=True, stop=True)
            XB = pcs.tile([128, 128], BF16, tag="XB", name="XB", bufs=2)
            nc.scalar.activation(out=XB, in_=psXB, func=AF.Copy)
            psZB = ppt.tile([128, 128], BF16, tag="ps", name="psZB")
            nc.tensor.transpose(psZB, XB, identb)
            ZB = pcs.tile([128, 128], BF16, tag="ZB", name="ZB", bufs=2)
            nc.vector.tensor_copy(ZB, psZB)

            # Tt = T^T = (I+Z4)(I+Z2)(I+Z) = X4I^T @ ZB; lhsT roles below then
            # give u = Tt^T vb = T vb and Gt = Kb^T Tt = (T Kb)^T.
            psT = ppt.tile([128, 128], F32, tag="ps", name="psT")
            nc.tensor.matmul(psT, X4I, ZB, start=True, stop=True)
            T = pcs.tile([128, 128], BF16, tag="T", name="T", bufs=2)
            nc.scalar.activation(out=T, in_=psT, func=AF.Copy)

            # --- v / k token-layout pair ---
            psVK = ppt.tile([128, 512], BF16, tag="ps", name="psVK")
            nc.tensor.transpose(psVK[:, 0:128], v0[:, ch], identb)
            nc.tensor.transpose(psVK[:, 128:256], v1[:, ch], identb)
            nc.tensor.transpose(psVK[:, 256:384], k0, identb)
            nc.tensor.transpose(psVK[:, 384:512], k1, identb)
            vb = pcm.tile([128, 256], BF16, tag="vb", name="vb", bufs=2)
            nc.vector.tensor_scalar(
                out=vb, in0=psVK[:, 0:256], scalar1=bk_t, scalar2=None, op0=OP.mult
            )
            ktok = pwin.tile([128, 256], BF16, tag="ktok", name="ktok")
            nc.scalar.activation(out=ktok, in_=psVK[:, 256:512], func=AF.Copy)

            # --- u = Tt vb  (W-chain seed) ---
            psU = ppt.tile([128, 256], F32, tag="ps", name="psU")
            nc.tensor.matmul(psU, T, vb, start=True, stop=True)
            u = pwin.tile([128, 256], BF16, tag="u", name="u")
            nc.scalar.activation(out=u, in_=psU, func=AF.Copy)

            # --- Gt = (Tt diag(nbk2) K)^T  [d, tok] ---
            Gt = None
            if i > 0:
                Kb = pcm.tile([128, 256], BF16, tag="Kb", name="Kb", bufs=2)
                nc.vector.tensor_scalar(
                    out=Kb, in0=psVK[:, 256:512], scalar1=nbk2_t, scalar2=None,
                    op0=OP.mult,
                )
                psGt = ppt.tile([128, 256], F32, tag="ps", name="psGt")
                nc.tensor.matmul(psGt[:, 0:128], Kb[:, 0:128], T,
                                 start=True, stop=True)
                nc.tensor.matmul(psGt[:, 128:256], Kb[:, 128:256], T,
                                 start=True, stop=True)
                Gt = pwin.tile([128, 256], BF16, tag="Gt", name="Gt")
                nc.scalar.activation(out=Gt, in_=psGt, func=AF.Copy)

            state[i] = dict(tok=tok, Hm=Hm, ktok=ktok, u=u, Gt=Gt)
            if dbg:
                nc.gpsimd.dma_start(out=dbg_d["tok"].ap()[:, i*4:(i+1)*4], in_=tok)
                nc.gpsimd.dma_start(out=dbg_d["X"].ap()[:, i*128:(i+1)*128], in_=X)
                nc.gpsimd.dma_start(out=dbg_d["T"].ap()[:, i*128:(i+1)*128], in_=T)
                nc.gpsimd.dma_start(out=dbg_d["u"].ap()[:, i*256:(i+1)*256], in_=u)
                nc.gpsimd.dma_start(
                    out=dbg_d["ktok"].ap()[:, i*256:(i+1)*256], in_=ktok)
                if i > 0:
                    nc.gpsimd.dma_start(
                        out=dbg_d["Gt"].ap()[:, i*256:(i+1)*256], in_=Gt)

        def emit_chain(i):
            ch = slice(i * C, (i + 1) * C)
            st = state[i]
            S_sb = None
            if i > 0:
                S_sb = pS.tile([128, 512], BF16, tag="S", name="S_sb")
                nc.scalar.activation(out=S_sb[:, 0:256], in_=psS0, func=AF.Copy)
                nc.scalar.activation(out=S_sb[:, 256:512], in_=psS1, func=AF.Copy)

            psW = ppw.tile([128, 256], F32, tag="cw", name="psW")
            if i > 0:
                nc.tensor.matmul(psW, identb, st["u"], start=True, stop=False)
                nc.tensor.matmul(
                    psW, st["Gt"][:, 0:128], S_sb[:, 0:256],
                    start=False, stop=False,
                )
                nc.tensor.matmul(
                    psW, st["Gt"][:, 128:256], S_sb[:, 256:512],
                    start=False, stop=True,
                )
            else:
                nc.tensor.matmul(psW, identb, st["u"], start=True, stop=True)
            w = pcm.tile([128, 256], BF16, tag="w", name="w", bufs=3)
            nc.vector.tensor_copy(w, psW)
            if dbg:
                nc.gpsimd.dma_start(out=dbg_d["w"].ap()[:, i*256:(i+1)*256], in_=w)
                if i > 0:
                    nc.gpsimd.dma_start(
                        out=dbg_d["S_sb"].ap()[:, i*512:(i+1)*512], in_=S_sb)

            # state update first so the chain keeps moving
            nc.tensor.matmul(
                psS0, st["ktok"][:, 0:128], w,
                start=(i == 0), stop=(i == nchunk - 1), skip_group_check=True,
            )
            nc.tensor.matmul(
                psS1, st["ktok"][:, 128:256], w,
                start=(i == 0), stop=(i == nchunk - 1), skip_group_check=True,
            )

            pso = ppw.tile([128, 256], F32, tag="cw", name="pso")
            if i > 0:
                nc.tensor.matmul(pso, kq0[:, 1, ch], S_sb[:, 0:256],
                                 start=True, stop=False)
                nc.tensor.matmul(pso, kq1[:, 1, ch], S_sb[:, 256:512],
                                 start=False, stop=False)
                nc.tensor.matmul(pso, st["Hm"], w, start=False, stop=True)
            else:
                nc.tensor.matmul(pso, st["Hm"], w, start=True, stop=True)
            st["pso"] = pso

        def emit_trailing(i):
            ch = slice(i * C, (i + 1) * C)
            st = state.pop(i)
            tok = st["tok"]
            pso = st["pso"]

            o_sb = pcm.tile([128, 256], BF16, tag="o_sb", name="o_sb", bufs=2)
            nc.vector.tensor_copy(o_sb, pso)
            if dbg:
                nc.gpsimd.dma_start(
                    out=dbg_d["o_sb"].ap()[:, i*256:(i+1)*256], in_=o_sb)
            psOT = ppt.tile([128, 256], BF16, tag="ps", name="psOT")
            nc.tensor.transpose(psOT[:, 0:128], o_sb[:, 0:128], identb)
            nc.tensor.transpose(psOT[:, 128:256], o_sb[:, 128:256], identb)
            ot = pcm.tile([128, 256], BF16, tag="ot", name="ot", bufs=2)
            nc.vector.tensor_copy(ot, psOT)

            # rms sums via matmul on squared ot
            sqot = pcm.tile([128, 256], BF16, tag="sqot", name="sqot", bufs=2)
            nc.vector.tensor_mul(sqot, ot, ot)
            psSums = ppt.tile([128, 4], F32, tag="ps", name="psSums")
            nc.tensor.matmul(psSums[:, 0:1], sqot[:, 0:128], onescol,
                             start=True, stop=False, skip_group_check=True)
            nc.tensor.matmul(psSums[:, 0:1], sqot[:, 128:256], onescol,
                             start=False, stop=True, skip_group_check=True)
            # rs_aq = aq/sqrt(aq^2/D * sums + eps5)
            rs = pcs.tile([128, 3], F32, tag="rs", name="rs", bufs=2)
            nc.scalar.activation(
                out=rs[:, 0:1], in_=psSums[:, 0:1], func=AF.Sqrt,
                scale=tok[:, 3:4], bias=eps5,
            )
            nc.vector.reciprocal(out=rs[:, 2:3], in_=rs[:, 0:1])
            nc.vector.tensor_mul(rs[:, 1:2], rs[:, 2:3], tok[:, 2:3])

            outbuf = pout.tile([128, HID], F32, tag="outbuf", name="outbuf")
            for hc in range(2):
                psop = ppt.tile([128, 512], F32, tag="ps", name="psop")
                nc.tensor.matmul(
                    psop, ot[:, 0:128], wo_sb[:, 0, hc * 512 : (hc + 1) * 512],
                    start=True, stop=False,
                )
                nc.tensor.matmul(
                    psop, ot[:, 128:256], wo_sb[:, 1, hc * 512 : (hc + 1) * 512],
                    start=False, stop=True,
                )
                dst = outbuf[:, hc * 512 : (hc + 1) * 512]
                if hc == 0:
                    nc.vector.tensor_scalar(
                        out=dst, in0=psop, scalar1=rs[:, 1:2], scalar2=None,
                        op0=OP.mult,
                    )
                else:
                    nc.scalar.activation(
                        out=dst, in_=psop, func=AF.Copy, scale=rs[:, 1:2]
                    )
            nc.gpsimd.dma_start(out=out_d.ap()[ch, :], in_=outbuf)

        LA = int(os.environ.get("KLOOKAHEAD", LOOKAHEAD))
        if LA == 0:
            for i in range(nchunk):
                emit_pass1(i)
                emit_chain(i)
                emit_trailing(i)
        else:
            for i in range(min(LA, nchunk)):
                emit_pass1(i)
            for i in range(nchunk):
                emit_chain(i)
                if i + LA < nchunk:
                    emit_pass1(i + LA)
                if i > 0:
                    emit_trailing(i - 1)
            emit_trailing(nchunk - 1)
        if dbg:
            nc.gpsimd.dma_start(
                out=dbg_d["kq0"].ap(), in_=kq0.rearrange("p a b -> p (a b)"))
            nc.gpsimd.dma_start(
                out=dbg_d["kq1"].ap(), in_=kq1.rearrange("p a b -> p (a b)"))
            nc.gpsimd.dma_start(out=dbg_d["v0"].ap(), in_=v0)
            nc.gpsimd.dma_start(out=dbg_d["v1"].ap(), in_=v1)
        dctx.close()

    nc.compile()
    return nc


def make_host_inputs(inputs, nchunk=S_FULL // C):
    S = nchunk * C
    bf = ml_dtypes.bfloat16
    hs = np.asarray(inputs["hidden_states"], np.float32)[:, :S, :]
    Wq, Wk, Wv = (np.asarray(inputs[k], np.float32) for k in ("Wq", "Wk", "Wv"))
    Wb = np.asarray(inputs["Wb"], np.float32)
    Wo = np.asarray(inputs["Wo"], np.float32)
    nw = np.asarray(inputs["norm_w"], np.float32)
    convs = {
        k: np.asarray(inputs[k], np.float32) for k in ("conv_q", "conv_k", "conv_v")
    }

    identb = np.eye(128, dtype=np.float32)
    onescol = np.ones((128, 1), np.float32)
    mlow = np.tril(np.ones((128, 128), np.float32), -1)
    mup = np.triu(np.ones((128, 128), np.float32), 0)

    def diag_pack(cw):
        out = np.zeros((128, 8 * 128), np.float32)
        for j in range(4):
            for dt_ in range(2):
                blk = np.diag(cw[dt_ * 128 : (dt_ + 1) * 128, j])
                out[:, (j * 2 + dt_) * 128 : (j * 2 + dt_ + 1) * 128] = blk
        return out

    def c(a, dt=bf):
        return np.ascontiguousarray(a).astype(dt)

    in_maps = []
    for core in range(8):
        b, h = core // 4, core % 4
        hsel = slice(h * D, (h + 1) * D)
        in_maps.append(
            {
                "xt": c(hs[b].T),
                "wq": c(Wq[:, hsel]),
                "wk": c(Wk[:, hsel]),
                "wv": c(Wv[:, hsel]),
                "wb": c(Wb[:, h : h + 1]),
                "wo": c(nw[:, None] * Wo[hsel, :]),
                "cdq": c(diag_pack(convs["conv_q"][hsel])),
                "cdk": c(diag_pack(convs["conv_k"][hsel])),
                "cdv": c(diag_pack(convs["conv_v"][hsel])),
                "identb": c(identb),
                "onescol": c(onescol),
                "mlow": c(mlow, np.float32),
                "mup": c(mup, np.float32),
            }
        )
    return in_maps


_NC_CACHE = {}


def _get_nc(nchunk):
    if nchunk not in _NC_CACHE:
        _NC_CACHE[nchunk] = build_nc(nchunk)
    return _NC_CACHE[nchunk]


def kernel(**inputs) -> np.ndarray:
    nchunk = S_FULL // C
    nc = _get_nc(nchunk)
    in_maps = make_host_inputs(inputs, nchunk)
    res = run_bass_kernel_spmd(nc, in_maps, core_ids=list(range(8)))
    S = nchunk * C
    out = np.zeros((B, S, HID), np.float32)
    for core in range(8):
        out[core // 4] += np.asarray(res.results[core]["out"], np.float32)
    return out
